# revision 20
# baseline (speedup 1.0000x reference)
"""nn_CNF Trainium2 Bass kernel — 8-core data-parallel.

Math (per batch row b of z (B, 32)):
    h      = tanh(z @ Wt.T + Bt)            (B, 64)
    dz_dt  = (h @ Ut) / 64                  (B, 32)
    dlogp  = (h^2 @ wu - sum(wu)) / 64      (B, 1)   [= -(1-h^2)@wu/64]
Wt/Ut/Bt come from a tiny RBF hypernetwork of t (computed on host),
wu = sum(Wt*Ut, axis=1).

Device layout (per core, 131072 rows, all matmuls bf16; mm1 uses an exact
hi/lo bf16 split of z and Wt — 3 accumulation passes, error ~1e-5):
  zT (128, 32768): partition 32q+d = z[q*32768 + n, d], q = batch quarter.
  64 superblocks of 512 columns (2048 rows each):
    mm1: 3 passes x 4 concurrent row/col-tiled K=32 matmuls -> PSUM (128,1024)
    ACT tanh + bias -> S_h (128, 1024) bf16
    DVE square      -> S_g (128, 1024) bf16
    mm2h: 2 block-diag K=128 M=64 matmuls -> dz.T packed (128, 512) PSUM
          ACT Copy evac -> SBUF (batched 4 sb) -> DMA out
    mm2g: 2 M=2 matmuls -> dlogp strips, partition-rotated over sb pairs,
          DVE evac once per 2 sb -> 4 small DMAs
Host unpacks dzT and applies the dlogp constant.
"""
import numpy as np

NCORES = 8
BATCH = 1048576
D = 32
WIDTH = 64
PER_CORE = BATCH // NCORES      # 131072
QUARTER = PER_CORE // 4         # 32768
SB = 512                        # superblock columns
NSB = QUARTER // SB             # 64
ZCHUNK = 4096                   # z-in DMA columns (8 superblocks, 1MB/tensor)
DZCHUNK = 4096                  # dz-out staging columns (8 superblocks, 2MB)
DLPGRP = 16                     # superblocks per dlogp flush (4 DMAs each)
ACTCOLS = 320                   # dz evac columns copied by ACT (rest on DVE)

_CACHE = {}


def _hypernet(t, centres, log_sigmas, lin_w, lin_b):
    # matches reference._rbf_linear in float32
    d = np.abs(t - centres[:, 0]) / np.exp(log_sigmas)
    phi = np.exp(-(d * d)).astype(np.float32)
    return phi @ lin_w.T + lin_b


def _split_bf16(x):
    """x (f32) -> (hi, lo) bf16 with hi + lo ~= x (error ~2^-17 rel)."""
    import ml_dtypes
    hi = x.astype(ml_dtypes.bfloat16)
    lo = (x - hi.astype(np.float32)).astype(ml_dtypes.bfloat16)
    return hi, lo


def _build(reps=1, mm1_passes=3, with_mm2g=True):
    import concourse.bass as bass
    import concourse.tile as tile
    from concourse import bacc, mybir
    from contextlib import ExitStack

    f32 = mybir.dt.float32
    bf16 = mybir.dt.bfloat16
    AF = mybir.ActivationFunctionType

    nc = bacc.Bacc("TRN2", target_bir_lowering=False, debug=False,
                   enable_asserts=True, num_devices=NCORES)

    za_d = nc.dram_tensor("z_il_a", [128, QUARTER], bf16, kind="ExternalInput").ap()
    zb_d = nc.dram_tensor("z_il_b", [128, QUARTER], bf16, kind="ExternalInput").ap()
    l1a_d = nc.dram_tensor("l1a", [128, 128], bf16, kind="ExternalInput").ap()
    l1b_d = nc.dram_tensor("l1b", [64, 128], bf16, kind="ExternalInput").ap()
    l2_d = nc.dram_tensor("lhsT2", [128, 128], bf16, kind="ExternalInput").ap()
    lg_d = nc.dram_tensor("lhsTg", [128, 2], bf16, kind="ExternalInput").ap()
    b_d = nc.dram_tensor("bias", [128, 1], f32, kind="ExternalInput").ap()

    dzT_d = nc.dram_tensor("dzT", [128, QUARTER], f32, kind="ExternalOutput").ap()
    dlp_d = nc.dram_tensor("dlp", [4, QUARTER], f32, kind="ExternalOutput").ap()

    with tile.TileContext(nc) as tc, ExitStack() as ctx:
        const = ctx.enter_context(tc.tile_pool(name="const", bufs=1))
        zin = ctx.enter_context(tc.tile_pool(name="zin", bufs=2))
        shp = ctx.enter_context(tc.tile_pool(name="sh", bufs=2))
        sgp = ctx.enter_context(tc.tile_pool(name="sg", bufs=2))
        dzp = ctx.enter_context(tc.tile_pool(name="dz", bufs=2))
        dlpp = ctx.enter_context(tc.tile_pool(name="dlp", bufs=2))
        # PSUM: p1 2 banks x2 + p2 1 bank x2 + pg 1 bank x1 = 7 banks
        p1p = ctx.enter_context(tc.tile_pool(name="p1", bufs=2, space="PSUM"))
        p2p = ctx.enter_context(tc.tile_pool(name="p2", bufs=2, space="PSUM"))
        pgp = ctx.enter_context(tc.tile_pool(name="pg", bufs=1, space="PSUM"))

        l1a_t = const.tile([128, 128], bf16)
        nc.sync.dma_start(l1a_t[:], l1a_d[:])
        l1b_t = const.tile([64, 128], bf16)
        nc.sync.dma_start(l1b_t[:], l1b_d[:])
        l2_t = const.tile([128, 128], bf16)
        nc.sync.dma_start(l2_t[:], l2_d[:])
        lg_t = const.tile([128, 2], bf16)
        nc.sync.dma_start(lg_t[:], lg_d[:])
        b_t = const.tile([128, 1], f32)
        nc.sync.dma_start(b_t[:], b_d[:])

        za_t = zb_t = None
        dzbig = None
        pg = None
        dlpacc = None

        def body():
            for m in range(NSB):
                emit_sb(m)

        def emit_sb(m):
            nonlocal za_t, zb_t, dzbig, pg, dlpacc
            zn = ZCHUNK // SB       # sbs per z chunk
            if m % zn == 0:
                za_t = zin.tile([128, ZCHUNK], bf16, tag="za")
                nc.sync.dma_start(za_t[:], za_d[:, m * SB:m * SB + ZCHUNK])
                zb_t = zin.tile([128, ZCHUNK], bf16, tag="zb")
                nc.sync.dma_start(zb_t[:], zb_d[:, m * SB:m * SB + ZCHUNK])
            c4 = (m % zn) * SB
            za = za_t[:, c4:c4 + SB]
            zb = zb_t[:, c4:c4 + SB]

            # mm1 per quarter-pair: z@W_hi (K=128, exact z) + z_hi@W_lo (K=64)
            p1 = p1p.tile([128, 2 * SB], f32)
            for bank, zt in ((0, za), (1, zb)):
                sl = p1[:, bank * SB:(bank + 1) * SB]
                nc.tensor.matmul(sl, l1a_t[:], zt,
                                 start=True, stop=(mm1_passes == 1))
                if mm1_passes > 1:
                    nc.tensor.matmul(sl, l1b_t[:], zt[0:64, :],
                                     start=False, stop=True)

            sh = shp.tile([128, 2 * SB], bf16)
            nc.scalar.activation(sh[:], p1[:], AF.Tanh, bias=b_t[:], scale=1.0)
            sg = sgp.tile([128, 2 * SB], bf16)
            nc.vector.tensor_mul(sg[:], sh[:], sh[:])

            # mm2h: dz.T packed (partition 32q+d)
            p2 = p2p.tile([128, SB], f32)
            nc.tensor.matmul(p2[0:64, :], l2_t[:, 0:64], sh[:, 0:SB],
                             start=True, stop=True, tile_position=(0, 0))
            nc.tensor.matmul(p2[64:128, :], l2_t[:, 64:128], sh[:, SB:2 * SB],
                             start=True, stop=True, tile_position=(0, 64))

            dn = DZCHUNK // SB
            if m % dn == 0:
                dzbig = dzp.tile([128, DZCHUNK], f32)
            cd = (m % dn) * SB
            # split the dz evacuation between ACT and DVE by columns
            nc.scalar.activation(dzbig[:, cd:cd + ACTCOLS], p2[:, 0:ACTCOLS],
                                 AF.Copy)
            nc.vector.tensor_copy(dzbig[:, cd + ACTCOLS:cd + SB],
                                  p2[:, ACTCOLS:SB])
            if m % dn == dn - 1:
                nc.sync.dma_start(dzT_d[:, (m - dn + 1) * SB:(m + 1) * SB],
                                  dzbig[:])

            # mm2g: dlogp strips; partition offset rotates over sb pairs
            if not with_mm2g:
                return
            off = 64 * (m % 2)
            if m % 2 == 0:
                pg = pgp.tile([128, SB], f32)
            nc.tensor.matmul(pg[off + 0:off + 2, :], lg_t[:], sg[:, 0:SB],
                             start=True, stop=True, tile_position=(0, off))
            nc.tensor.matmul(pg[off + 32:off + 34, :], lg_t[:], sg[:, SB:2 * SB],
                             start=True, stop=True, tile_position=(0, off + 32))
            if m % DLPGRP == 0:
                dlpacc = dlpp.tile([128, DLPGRP * SB // 2], f32)
            if m % 2 == 1:
                k = (m % DLPGRP) // 2
                nc.vector.tensor_copy(dlpacc[0:98, k * SB:(k + 1) * SB],
                                      pg[0:98, :])
            if m % DLPGRP == DLPGRP - 1:
                base = m - DLPGRP + 1
                np_ = DLPGRP // 2      # sb pairs in this flush group
                # dlpacc[{0,1},   k*SB+n] = dlogp q0/q1, sb base+2k (even)
                # dlpacc[{32,33}, k*SB+n] = dlogp q2/q3, sb base+2k
                # dlpacc[{64,65}, k*SB+n] = dlogp q0/q1, sb base+2k+1 (odd)
                # dlpacc[{96,97}, k*SB+n] = dlogp q2/q3, sb base+2k+1
                for (p0, r0, o0) in ((0, 0, 0), (32, 2, 0), (64, 0, 1),
                                     (96, 2, 1)):
                    src = dlpacc[p0:p0 + 2, :].rearrange(
                        "p (k n) -> p k n", n=SB)
                    dst_ap = dlp_d[r0:r0 + 2, base * SB:(base + DLPGRP) * SB] \
                        .rearrange("p (k two n) -> p two k n", two=2, n=SB)[:, o0]
                    nc.sync.dma_start(dst_ap, src)

        if reps == 1:
            body()
        else:
            # timing-only variant: hardware loop repeating identical work
            with tc.For_i(0, reps, 1):
                body()

    nc.compile()
    return nc


def _get_nc(reps=1):
    key = f"nc{reps}"
    if key not in _CACHE:
        _CACHE[key] = _build(reps)
    return _CACHE[key]


def _prep_inputs(t, z,
                 W_centres, W_log_sigmas, W_lin_w, W_lin_b,
                 U_centres, U_log_sigmas, U_lin_w, U_lin_b,
                 B_centres, B_log_sigmas, B_lin_w, B_lin_b):
    t = np.asarray(t, np.float32)
    Wt = _hypernet(t, np.asarray(W_centres, np.float32),
                   np.asarray(W_log_sigmas, np.float32),
                   np.asarray(W_lin_w, np.float32),
                   np.asarray(W_lin_b, np.float32)).reshape(WIDTH, D)
    Ut = _hypernet(t, np.asarray(U_centres, np.float32),
                   np.asarray(U_log_sigmas, np.float32),
                   np.asarray(U_lin_w, np.float32),
                   np.asarray(U_lin_b, np.float32)).reshape(WIDTH, D)
    Bt = _hypernet(t, np.asarray(B_centres, np.float32),
                   np.asarray(B_log_sigmas, np.float32),
                   np.asarray(B_lin_w, np.float32),
                   np.asarray(B_lin_b, np.float32))
    wu = np.sum(Wt * Ut, axis=1)

    U64 = (Ut / np.float32(WIDTH)).astype(np.float32)          # (64, 32)
    wu64 = (wu / np.float32(WIDTH)).astype(np.float32)         # (64,)

    import ml_dtypes
    bf = ml_dtypes.bfloat16

    WtT_hi, WtT_lo = _split_bf16(Wt.T.astype(np.float32))      # (32, 64) each
    # l1a (128, 128): pass1 = (z_hi + z_lo) @ W_hi for a quarter pair whose
    # rhs partitions are [q_hi(32), q'_hi(32), q_lo(32), q'_lo(32)]
    l1a = np.zeros((128, 128), np.float32)
    l1a[0:32, 0:64] = WtT_hi
    l1a[32:64, 64:128] = WtT_hi
    l1a[64:96, 0:64] = WtT_hi
    l1a[96:128, 64:128] = WtT_hi
    l1a = l1a.astype(bf)
    # l1b (64, 128): pass2 = z_hi @ W_lo
    l1b = np.zeros((64, 128), np.float32)
    l1b[0:32, 0:64] = WtT_lo
    l1b[32:64, 64:128] = WtT_lo
    l1b = l1b.astype(bf)

    blk = np.zeros((128, 64), np.float32)
    blk[0:64, 0:32] = U64
    blk[64:128, 32:64] = U64
    lhsT2 = np.concatenate([blk, blk], axis=1).astype(bf)      # (128, 128)
    lhsTg = np.zeros((128, 2), np.float32)
    lhsTg[0:64, 0] = wu64
    lhsTg[64:128, 1] = wu64
    lhsTg = lhsTg.astype(bf)
    bias = np.concatenate([Bt, Bt]).reshape(128, 1).astype(np.float32)

    z = np.ascontiguousarray(np.asarray(z, np.float32))
    # per-core packed transpose: (8, 4, 32768, 32) -> (8, 4, 32, 32768)
    zT = z.reshape(NCORES, 4, QUARTER, D).transpose(0, 1, 3, 2) \
          .reshape(NCORES, 128, QUARTER)
    z_hi, z_lo = _split_bf16(zT)
    # interleaved pair layouts: [q_hi, q'_hi, q_lo, q'_lo]
    z_il_a = np.concatenate(
        [z_hi[:, 0:64], z_lo[:, 0:64]], axis=1)                # (8, 128, Q)
    z_il_b = np.concatenate(
        [z_hi[:, 64:128], z_lo[:, 64:128]], axis=1)
    z_il_a = np.ascontiguousarray(z_il_a)
    z_il_b = np.ascontiguousarray(z_il_b)

    in_maps = [
        dict(z_il_a=z_il_a[c], z_il_b=z_il_b[c], l1a=l1a, l1b=l1b,
             lhsT2=lhsT2, lhsTg=lhsTg, bias=bias)
        for c in range(NCORES)
    ]
    return in_maps, wu


def _postprocess(results, wu):
    const = np.float32(np.sum(wu) / np.float32(WIDTH))
    dz_parts, dlp_parts = [], []
    for c in range(NCORES):
        dzT = results[c]["dzT"]                       # (128, 32768)
        dz = dzT.reshape(4, D, QUARTER).transpose(0, 2, 1).reshape(PER_CORE, D)
        dz_parts.append(dz)
        dlp = results[c]["dlp"].reshape(PER_CORE) - const
        dlp_parts.append(dlp)
    dz_dt = np.ascontiguousarray(np.concatenate(dz_parts, axis=0), dtype=np.float32)
    dlogp = np.concatenate(dlp_parts, axis=0).reshape(BATCH, 1).astype(np.float32)
    return dz_dt, dlogp


def kernel(t, z, logp_z, **params):
    from concourse.bass_utils import run_bass_kernel_spmd

    nc = _get_nc()
    in_maps, wu = _prep_inputs(t, z, **params)
    res = run_bass_kernel_spmd(nc, in_maps, core_ids=list(range(NCORES)))
    return _postprocess(res.results, wu)


# revision 21
# speedup vs baseline: 1.9922x; 1.9922x over previous
"""nn_CNF Trainium2 Bass kernel — 8-core data-parallel.

Math (per batch row b of z (B, 32)):
    h      = tanh(z @ Wt.T + Bt)            (B, 64)
    dz_dt  = (h @ Ut) / 64                  (B, 32)
    dlogp  = (h^2 @ wu - sum(wu)) / 64      (B, 1)   [= -(1-h^2)@wu/64]
Wt/Ut/Bt come from a tiny RBF hypernetwork of t (computed on host),
wu = sum(Wt*Ut, axis=1).

Device layout (per core, 131072 rows, all matmuls bf16; mm1 uses an exact
hi/lo bf16 split of z and Wt — 3 accumulation passes, error ~1e-5):
  zT (128, 32768): partition 32q+d = z[q*32768 + n, d], q = batch quarter.
  64 superblocks of 512 columns (2048 rows each):
    mm1: 3 passes x 4 concurrent row/col-tiled K=32 matmuls -> PSUM (128,1024)
    ACT tanh + bias -> S_h (128, 1024) bf16
    DVE square      -> S_g (128, 1024) bf16
    mm2h: 2 block-diag K=128 M=64 matmuls -> dz.T packed (128, 512) PSUM
          ACT Copy evac -> SBUF (batched 4 sb) -> DMA out
    mm2g: 2 M=2 matmuls -> dlogp strips, partition-rotated over sb pairs,
          DVE evac once per 2 sb -> 4 small DMAs
Host unpacks dzT and applies the dlogp constant.
"""
import numpy as np

NCORES = 8
BATCH = 1048576
D = 32
WIDTH = 64
PER_CORE = BATCH // NCORES      # 131072
QUARTER = PER_CORE // 4         # 32768
SB = 512                        # superblock columns
NSB = QUARTER // SB             # 64
ZCHUNK = 4096                   # z-in DMA columns (8 superblocks, 1MB/tensor)
DZCHUNK = 4096                  # dz-out staging columns (8 superblocks, 2MB)
DLPGRP = 16                     # superblocks per dlogp flush (4 DMAs each)
ACTCOLS = 320                   # dz evac columns copied by ACT (rest on DVE)

_CACHE = {}


def _hypernet(t, centres, log_sigmas, lin_w, lin_b):
    # matches reference._rbf_linear in float32
    d = np.abs(t - centres[:, 0]) / np.exp(log_sigmas)
    phi = np.exp(-(d * d)).astype(np.float32)
    return phi @ lin_w.T + lin_b


def _split_bf16(x):
    """x (f32) -> (hi, lo) bf16 with hi + lo ~= x (error ~2^-17 rel)."""
    import ml_dtypes
    hi = x.astype(ml_dtypes.bfloat16)
    lo = (x - hi.astype(np.float32)).astype(ml_dtypes.bfloat16)
    return hi, lo


def _build(reps=1, mm1_passes=3, with_mm2g=True):
    import concourse.bass as bass
    import concourse.tile as tile
    from concourse import bacc, mybir
    from contextlib import ExitStack

    f32 = mybir.dt.float32
    bf16 = mybir.dt.bfloat16
    AF = mybir.ActivationFunctionType

    nc = bacc.Bacc("TRN2", target_bir_lowering=False, debug=False,
                   enable_asserts=True, num_devices=NCORES)

    za_d = nc.dram_tensor("z_il_a", [128, QUARTER], bf16, kind="ExternalInput").ap()
    zb_d = nc.dram_tensor("z_il_b", [128, QUARTER], bf16, kind="ExternalInput").ap()
    l1a_d = nc.dram_tensor("l1a", [128, 128], bf16, kind="ExternalInput").ap()
    l1b_d = nc.dram_tensor("l1b", [64, 128], bf16, kind="ExternalInput").ap()
    l2_d = nc.dram_tensor("lhsT2", [128, 128], bf16, kind="ExternalInput").ap()
    lg_d = nc.dram_tensor("lhsTg", [128, 2], bf16, kind="ExternalInput").ap()
    b_d = nc.dram_tensor("bias", [128, 1], f32, kind="ExternalInput").ap()

    dzT_d = nc.dram_tensor("dzT", [128, QUARTER], f32, kind="ExternalOutput").ap()
    dlp_d = nc.dram_tensor("dlp", [4, QUARTER], f32, kind="ExternalOutput").ap()

    with tile.TileContext(nc) as tc, ExitStack() as ctx:
        const = ctx.enter_context(tc.tile_pool(name="const", bufs=1))
        zin = ctx.enter_context(tc.tile_pool(name="zin", bufs=2))
        shp = ctx.enter_context(tc.tile_pool(name="sh", bufs=2))
        sgp = ctx.enter_context(tc.tile_pool(name="sg", bufs=2))
        dzp = ctx.enter_context(tc.tile_pool(name="dz", bufs=2))
        dlpp = ctx.enter_context(tc.tile_pool(name="dlp", bufs=2))
        # PSUM: p1 2 banks x2 + p2 1 bank x2 + pg 1 bank x1 = 7 banks
        p1p = ctx.enter_context(tc.tile_pool(name="p1", bufs=2, space="PSUM"))
        p2p = ctx.enter_context(tc.tile_pool(name="p2", bufs=2, space="PSUM"))
        pgp = ctx.enter_context(tc.tile_pool(name="pg", bufs=1, space="PSUM"))

        l1a_t = const.tile([128, 128], bf16)
        nc.sync.dma_start(l1a_t[:], l1a_d[:])
        l1b_t = const.tile([64, 128], bf16)
        nc.sync.dma_start(l1b_t[:], l1b_d[:])
        l2_t = const.tile([128, 128], bf16)
        nc.sync.dma_start(l2_t[:], l2_d[:])
        lg_t = const.tile([128, 2], bf16)
        nc.sync.dma_start(lg_t[:], lg_d[:])
        b_t = const.tile([128, 1], f32)
        nc.sync.dma_start(b_t[:], b_d[:])

        za_t = zb_t = None
        dzbig = None
        pg = None
        dlpacc = None

        p1s = {}

        def body():
            # software-pipelined: mm1(m+1) is emitted before the tail of m so
            # the in-order PE fills its tanh/square wait with the next mm1.
            emit_mm1(0)
            for m in range(NSB):
                if m + 1 < NSB:
                    emit_mm1(m + 1)
                emit_tail(m)

        def emit_mm1(m):
            nonlocal za_t, zb_t
            zn = ZCHUNK // SB       # sbs per z chunk
            if m % zn == 0:
                za_t = zin.tile([128, ZCHUNK], bf16, tag="za")
                nc.sync.dma_start(za_t[:], za_d[:, m * SB:m * SB + ZCHUNK])
                zb_t = zin.tile([128, ZCHUNK], bf16, tag="zb")
                nc.sync.dma_start(zb_t[:], zb_d[:, m * SB:m * SB + ZCHUNK])
            c4 = (m % zn) * SB
            za = za_t[:, c4:c4 + SB]
            zb = zb_t[:, c4:c4 + SB]

            # mm1 per quarter-pair: z@W_hi (K=128, exact z) + z_hi@W_lo (K=64)
            p1 = p1p.tile([128, 2 * SB], f32)
            p1s[m] = p1
            for bank, zt in ((0, za), (1, zb)):
                sl = p1[:, bank * SB:(bank + 1) * SB]
                nc.tensor.matmul(sl, l1a_t[:], zt,
                                 start=True, stop=(mm1_passes == 1))
                if mm1_passes > 1:
                    nc.tensor.matmul(sl, l1b_t[:], zt[0:64, :],
                                     start=False, stop=True)

        def emit_tail(m):
            nonlocal dzbig, pg, dlpacc
            p1 = p1s.pop(m)

            sh = shp.tile([128, 2 * SB], bf16)
            nc.scalar.activation(sh[:], p1[:], AF.Tanh, bias=b_t[:], scale=1.0)
            sg = sgp.tile([128, 2 * SB], bf16)
            nc.vector.tensor_mul(sg[:], sh[:], sh[:])

            # mm2h: dz.T packed (partition 32q+d)
            p2 = p2p.tile([128, SB], f32)
            nc.tensor.matmul(p2[0:64, :], l2_t[:, 0:64], sh[:, 0:SB],
                             start=True, stop=True, tile_position=(0, 0))
            nc.tensor.matmul(p2[64:128, :], l2_t[:, 64:128], sh[:, SB:2 * SB],
                             start=True, stop=True, tile_position=(0, 64))

            dn = DZCHUNK // SB
            if m % dn == 0:
                dzbig = dzp.tile([128, DZCHUNK], f32)
            cd = (m % dn) * SB
            # split the dz evacuation between ACT and DVE by columns
            nc.scalar.activation(dzbig[:, cd:cd + ACTCOLS], p2[:, 0:ACTCOLS],
                                 AF.Copy)
            nc.vector.tensor_copy(dzbig[:, cd + ACTCOLS:cd + SB],
                                  p2[:, ACTCOLS:SB])
            if m % dn == dn - 1:
                nc.sync.dma_start(dzT_d[:, (m - dn + 1) * SB:(m + 1) * SB],
                                  dzbig[:])

            # mm2g: dlogp strips; partition offset rotates over sb pairs
            if not with_mm2g:
                return
            off = 64 * (m % 2)
            if m % 2 == 0:
                pg = pgp.tile([128, SB], f32)
            nc.tensor.matmul(pg[off + 0:off + 2, :], lg_t[:], sg[:, 0:SB],
                             start=True, stop=True, tile_position=(0, off))
            nc.tensor.matmul(pg[off + 32:off + 34, :], lg_t[:], sg[:, SB:2 * SB],
                             start=True, stop=True, tile_position=(0, off + 32))
            if m % DLPGRP == 0:
                dlpacc = dlpp.tile([128, DLPGRP * SB // 2], f32)
            if m % 2 == 1:
                k = (m % DLPGRP) // 2
                nc.vector.tensor_copy(dlpacc[0:98, k * SB:(k + 1) * SB],
                                      pg[0:98, :])
            if m % DLPGRP == DLPGRP - 1:
                base = m - DLPGRP + 1
                np_ = DLPGRP // 2      # sb pairs in this flush group
                # dlpacc[{0,1},   k*SB+n] = dlogp q0/q1, sb base+2k (even)
                # dlpacc[{32,33}, k*SB+n] = dlogp q2/q3, sb base+2k
                # dlpacc[{64,65}, k*SB+n] = dlogp q0/q1, sb base+2k+1 (odd)
                # dlpacc[{96,97}, k*SB+n] = dlogp q2/q3, sb base+2k+1
                for (p0, r0, o0) in ((0, 0, 0), (32, 2, 0), (64, 0, 1),
                                     (96, 2, 1)):
                    src = dlpacc[p0:p0 + 2, :].rearrange(
                        "p (k n) -> p k n", n=SB)
                    dst_ap = dlp_d[r0:r0 + 2, base * SB:(base + DLPGRP) * SB] \
                        .rearrange("p (k two n) -> p two k n", two=2, n=SB)[:, o0]
                    nc.sync.dma_start(dst_ap, src)

        if reps == 1:
            body()
        else:
            # timing-only variant: hardware loop repeating identical work
            with tc.For_i(0, reps, 1):
                body()

    nc.compile()
    return nc


def _get_nc(reps=1):
    key = f"nc{reps}"
    if key not in _CACHE:
        _CACHE[key] = _build(reps)
    return _CACHE[key]


def _prep_inputs(t, z,
                 W_centres, W_log_sigmas, W_lin_w, W_lin_b,
                 U_centres, U_log_sigmas, U_lin_w, U_lin_b,
                 B_centres, B_log_sigmas, B_lin_w, B_lin_b):
    t = np.asarray(t, np.float32)
    Wt = _hypernet(t, np.asarray(W_centres, np.float32),
                   np.asarray(W_log_sigmas, np.float32),
                   np.asarray(W_lin_w, np.float32),
                   np.asarray(W_lin_b, np.float32)).reshape(WIDTH, D)
    Ut = _hypernet(t, np.asarray(U_centres, np.float32),
                   np.asarray(U_log_sigmas, np.float32),
                   np.asarray(U_lin_w, np.float32),
                   np.asarray(U_lin_b, np.float32)).reshape(WIDTH, D)
    Bt = _hypernet(t, np.asarray(B_centres, np.float32),
                   np.asarray(B_log_sigmas, np.float32),
                   np.asarray(B_lin_w, np.float32),
                   np.asarray(B_lin_b, np.float32))
    wu = np.sum(Wt * Ut, axis=1)

    U64 = (Ut / np.float32(WIDTH)).astype(np.float32)          # (64, 32)
    wu64 = (wu / np.float32(WIDTH)).astype(np.float32)         # (64,)

    import ml_dtypes
    bf = ml_dtypes.bfloat16

    WtT_hi, WtT_lo = _split_bf16(Wt.T.astype(np.float32))      # (32, 64) each
    # l1a (128, 128): pass1 = (z_hi + z_lo) @ W_hi for a quarter pair whose
    # rhs partitions are [q_hi(32), q'_hi(32), q_lo(32), q'_lo(32)]
    l1a = np.zeros((128, 128), np.float32)
    l1a[0:32, 0:64] = WtT_hi
    l1a[32:64, 64:128] = WtT_hi
    l1a[64:96, 0:64] = WtT_hi
    l1a[96:128, 64:128] = WtT_hi
    l1a = l1a.astype(bf)
    # l1b (64, 128): pass2 = z_hi @ W_lo
    l1b = np.zeros((64, 128), np.float32)
    l1b[0:32, 0:64] = WtT_lo
    l1b[32:64, 64:128] = WtT_lo
    l1b = l1b.astype(bf)

    blk = np.zeros((128, 64), np.float32)
    blk[0:64, 0:32] = U64
    blk[64:128, 32:64] = U64
    lhsT2 = np.concatenate([blk, blk], axis=1).astype(bf)      # (128, 128)
    lhsTg = np.zeros((128, 2), np.float32)
    lhsTg[0:64, 0] = wu64
    lhsTg[64:128, 1] = wu64
    lhsTg = lhsTg.astype(bf)
    bias = np.concatenate([Bt, Bt]).reshape(128, 1).astype(np.float32)

    z = np.ascontiguousarray(np.asarray(z, np.float32))
    # per-core packed transpose: (8, 4, 32768, 32) -> (8, 4, 32, 32768)
    zT = z.reshape(NCORES, 4, QUARTER, D).transpose(0, 1, 3, 2) \
          .reshape(NCORES, 128, QUARTER)
    z_hi, z_lo = _split_bf16(zT)
    # interleaved pair layouts: [q_hi, q'_hi, q_lo, q'_lo]
    z_il_a = np.concatenate(
        [z_hi[:, 0:64], z_lo[:, 0:64]], axis=1)                # (8, 128, Q)
    z_il_b = np.concatenate(
        [z_hi[:, 64:128], z_lo[:, 64:128]], axis=1)
    z_il_a = np.ascontiguousarray(z_il_a)
    z_il_b = np.ascontiguousarray(z_il_b)

    in_maps = [
        dict(z_il_a=z_il_a[c], z_il_b=z_il_b[c], l1a=l1a, l1b=l1b,
             lhsT2=lhsT2, lhsTg=lhsTg, bias=bias)
        for c in range(NCORES)
    ]
    return in_maps, wu


def _postprocess(results, wu):
    const = np.float32(np.sum(wu) / np.float32(WIDTH))
    dz_parts, dlp_parts = [], []
    for c in range(NCORES):
        dzT = results[c]["dzT"]                       # (128, 32768)
        dz = dzT.reshape(4, D, QUARTER).transpose(0, 2, 1).reshape(PER_CORE, D)
        dz_parts.append(dz)
        dlp = results[c]["dlp"].reshape(PER_CORE) - const
        dlp_parts.append(dlp)
    dz_dt = np.ascontiguousarray(np.concatenate(dz_parts, axis=0), dtype=np.float32)
    dlogp = np.concatenate(dlp_parts, axis=0).reshape(BATCH, 1).astype(np.float32)
    return dz_dt, dlogp


def kernel(t, z, logp_z, **params):
    from concourse.bass_utils import run_bass_kernel_spmd

    nc = _get_nc()
    in_maps, wu = _prep_inputs(t, z, **params)
    res = run_bass_kernel_spmd(nc, in_maps, core_ids=list(range(NCORES)))
    return _postprocess(res.results, wu)


# revision 22
# speedup vs baseline: 3.2386x; 1.6256x over previous
"""nn_CNF Trainium2 Bass kernel — 8-core data-parallel.

Math (per batch row b of z (B, 32)):
    h      = tanh(z @ Wt.T + Bt)            (B, 64)
    dz_dt  = (h @ Ut) / 64                  (B, 32)
    dlogp  = (h^2 @ wu - sum(wu)) / 64      (B, 1)   [= -(1-h^2)@wu/64]
Wt/Ut/Bt come from a tiny RBF hypernetwork of t (computed on host),
wu = sum(Wt*Ut, axis=1).

Device layout (per core, 131072 rows, all matmuls bf16; mm1 uses an exact
hi/lo bf16 split of z and Wt — 3 accumulation passes, error ~1e-5):
  zT (128, 32768): partition 32q+d = z[q*32768 + n, d], q = batch quarter.
  64 superblocks of 512 columns (2048 rows each):
    mm1: 3 passes x 4 concurrent row/col-tiled K=32 matmuls -> PSUM (128,1024)
    ACT tanh + bias -> S_h (128, 1024) bf16
    DVE square      -> S_g (128, 1024) bf16
    mm2h: 2 block-diag K=128 M=64 matmuls -> dz.T packed (128, 512) PSUM
          ACT Copy evac -> SBUF (batched 4 sb) -> DMA out
    mm2g: 2 M=2 matmuls -> dlogp strips, partition-rotated over sb pairs,
          DVE evac once per 2 sb -> 4 small DMAs
Host unpacks dzT and applies the dlogp constant.
"""
import numpy as np

NCORES = 8
BATCH = 1048576
D = 32
WIDTH = 64
PER_CORE = BATCH // NCORES      # 131072
QUARTER = PER_CORE // 4         # 32768
SB = 512                        # superblock columns
NSB = QUARTER // SB             # 64
ZCHUNK = 4096                   # z-in DMA columns (8 superblocks, 1MB/tensor)
DZCHUNK = 4096                  # dz-out staging columns (8 superblocks, 2MB)
DLPGRP = 16                     # superblocks per dlogp flush (4 DMAs each)
ACTCOLS = 320                   # dz evac columns copied by ACT (rest on DVE)

_CACHE = {}


def _hypernet(t, centres, log_sigmas, lin_w, lin_b):
    # matches reference._rbf_linear in float32
    d = np.abs(t - centres[:, 0]) / np.exp(log_sigmas)
    phi = np.exp(-(d * d)).astype(np.float32)
    return phi @ lin_w.T + lin_b


def _split_bf16(x):
    """x (f32) -> (hi, lo) bf16 with hi + lo ~= x (error ~2^-17 rel)."""
    import ml_dtypes
    hi = x.astype(ml_dtypes.bfloat16)
    lo = (x - hi.astype(np.float32)).astype(ml_dtypes.bfloat16)
    return hi, lo


def _build(reps=1, mm1_passes=3, with_mm2g=True):
    import concourse.bass as bass
    import concourse.tile as tile
    from concourse import bacc, mybir
    from contextlib import ExitStack

    f32 = mybir.dt.float32
    bf16 = mybir.dt.bfloat16
    fp16 = mybir.dt.float16
    AF = mybir.ActivationFunctionType

    nc = bacc.Bacc("TRN2", target_bir_lowering=False, debug=False,
                   enable_asserts=True, num_devices=NCORES)

    za_d = nc.dram_tensor("z_il_a", [128, QUARTER], bf16, kind="ExternalInput").ap()
    zb_d = nc.dram_tensor("z_il_b", [128, QUARTER], bf16, kind="ExternalInput").ap()
    l1a_d = nc.dram_tensor("l1a", [128, 128], bf16, kind="ExternalInput").ap()
    l1b_d = nc.dram_tensor("l1b", [64, 128], bf16, kind="ExternalInput").ap()
    l2_d = nc.dram_tensor("lhsT2", [128, 128], fp16, kind="ExternalInput").ap()
    lg_d = nc.dram_tensor("lhsTg", [128, 2], fp16, kind="ExternalInput").ap()
    b_d = nc.dram_tensor("bias", [128, 1], f32, kind="ExternalInput").ap()

    dzT_d = nc.dram_tensor("dzT", [128, QUARTER], f32, kind="ExternalOutput").ap()
    dlp_d = nc.dram_tensor("dlp", [4, QUARTER], f32, kind="ExternalOutput").ap()

    with tile.TileContext(nc) as tc, ExitStack() as ctx:
        const = ctx.enter_context(tc.tile_pool(name="const", bufs=1))
        zin = ctx.enter_context(tc.tile_pool(name="zin", bufs=2))
        shp = ctx.enter_context(tc.tile_pool(name="sh", bufs=2))
        sgp = ctx.enter_context(tc.tile_pool(name="sg", bufs=2))
        dzp = ctx.enter_context(tc.tile_pool(name="dz", bufs=2))
        dlpp = ctx.enter_context(tc.tile_pool(name="dlp", bufs=2))
        # PSUM: p1 2 banks x2 + p2 1 bank x2 + pg 1 bank x1 = 7 banks
        p1p = ctx.enter_context(tc.tile_pool(name="p1", bufs=2, space="PSUM"))
        p2p = ctx.enter_context(tc.tile_pool(name="p2", bufs=2, space="PSUM"))
        pgp = ctx.enter_context(tc.tile_pool(name="pg", bufs=1, space="PSUM"))

        l1a_t = const.tile([128, 128], bf16)
        nc.sync.dma_start(l1a_t[:], l1a_d[:])
        l1b_t = const.tile([64, 128], bf16)
        nc.sync.dma_start(l1b_t[:], l1b_d[:])
        l2_t = const.tile([128, 128], fp16)
        nc.sync.dma_start(l2_t[:], l2_d[:])
        lg_t = const.tile([128, 2], fp16)
        nc.sync.dma_start(lg_t[:], lg_d[:])
        b_t = const.tile([128, 1], f32)
        nc.sync.dma_start(b_t[:], b_d[:])

        za_t = zb_t = None
        dzbig = None
        pg = None
        dlpacc = None

        p1s = {}

        def body():
            # software-pipelined: mm1(m+1) is emitted before the tail of m so
            # the in-order PE fills its tanh/square wait with the next mm1.
            emit_mm1(0)
            for m in range(NSB):
                if m + 1 < NSB:
                    emit_mm1(m + 1)
                emit_tail(m)

        def emit_mm1(m):
            nonlocal za_t, zb_t
            zn = ZCHUNK // SB       # sbs per z chunk
            if m % zn == 0:
                za_t = zin.tile([128, ZCHUNK], bf16, tag="za")
                nc.scalar.dma_start(za_t[:], za_d[:, m * SB:m * SB + ZCHUNK])
                zb_t = zin.tile([128, ZCHUNK], bf16, tag="zb")
                nc.scalar.dma_start(zb_t[:], zb_d[:, m * SB:m * SB + ZCHUNK])
            c4 = (m % zn) * SB
            za = za_t[:, c4:c4 + SB]
            zb = zb_t[:, c4:c4 + SB]

            # mm1 per quarter-pair: z@W_hi (K=128, exact z) + z_hi@W_lo (K=64)
            p1 = p1p.tile([128, 2 * SB], f32)
            p1s[m] = p1
            for bank, zt in ((0, za), (1, zb)):
                sl = p1[:, bank * SB:(bank + 1) * SB]
                nc.tensor.matmul(sl, l1a_t[:], zt,
                                 start=True, stop=(mm1_passes == 1))
                if mm1_passes > 1:
                    nc.tensor.matmul(sl, l1b_t[:], zt[0:64, :],
                                     start=False, stop=True)

        def emit_tail(m):
            nonlocal dzbig, pg, dlpacc
            p1 = p1s.pop(m)

            sh = shp.tile([128, 2 * SB], fp16)
            nc.scalar.activation(sh[:], p1[:], AF.Tanh, bias=b_t[:], scale=1.0)
            sg = sgp.tile([128, 2 * SB], fp16)
            nc.vector.tensor_mul(sg[:], sh[:], sh[:])

            # mm2h: dz.T packed (partition 32q+d)
            p2 = p2p.tile([128, SB], f32)
            nc.tensor.matmul(p2[0:64, :], l2_t[:, 0:64], sh[:, 0:SB],
                             start=True, stop=True, tile_position=(0, 0))
            nc.tensor.matmul(p2[64:128, :], l2_t[:, 64:128], sh[:, SB:2 * SB],
                             start=True, stop=True, tile_position=(0, 64))

            dn = DZCHUNK // SB
            if m % dn == 0:
                dzbig = dzp.tile([128, DZCHUNK], f32)
            cd = (m % dn) * SB
            # split the dz evacuation between ACT and DVE by columns
            nc.scalar.activation(dzbig[:, cd:cd + ACTCOLS], p2[:, 0:ACTCOLS],
                                 AF.Copy)
            nc.vector.tensor_copy(dzbig[:, cd + ACTCOLS:cd + SB],
                                  p2[:, ACTCOLS:SB])
            if m % dn == dn - 1:
                nc.sync.dma_start(dzT_d[:, (m - dn + 1) * SB:(m + 1) * SB],
                                  dzbig[:])

            # mm2g: dlogp strips; partition offset rotates over sb pairs
            if not with_mm2g:
                return
            off = 64 * (m % 2)
            if m % 2 == 0:
                pg = pgp.tile([128, SB], f32)
            nc.tensor.matmul(pg[off + 0:off + 2, :], lg_t[:], sg[:, 0:SB],
                             start=True, stop=True, tile_position=(0, off))
            nc.tensor.matmul(pg[off + 32:off + 34, :], lg_t[:], sg[:, SB:2 * SB],
                             start=True, stop=True, tile_position=(0, off + 32))
            if m % DLPGRP == 0:
                dlpacc = dlpp.tile([128, DLPGRP * SB // 2], f32)
            if m % 2 == 1:
                k = (m % DLPGRP) // 2
                nc.vector.tensor_copy(dlpacc[0:98, k * SB:(k + 1) * SB],
                                      pg[0:98, :])
            if m % DLPGRP == DLPGRP - 1:
                base = m - DLPGRP + 1
                np_ = DLPGRP // 2      # sb pairs in this flush group
                # dlpacc[{0,1},   k*SB+n] = dlogp q0/q1, sb base+2k (even)
                # dlpacc[{32,33}, k*SB+n] = dlogp q2/q3, sb base+2k
                # dlpacc[{64,65}, k*SB+n] = dlogp q0/q1, sb base+2k+1 (odd)
                # dlpacc[{96,97}, k*SB+n] = dlogp q2/q3, sb base+2k+1
                for (p0, r0, o0) in ((0, 0, 0), (32, 2, 0), (64, 0, 1),
                                     (96, 2, 1)):
                    src = dlpacc[p0:p0 + 2, :].rearrange(
                        "p (k n) -> p k n", n=SB)
                    dst_ap = dlp_d[r0:r0 + 2, base * SB:(base + DLPGRP) * SB] \
                        .rearrange("p (k two n) -> p two k n", two=2, n=SB)[:, o0]
                    nc.sync.dma_start(dst_ap, src)

        if reps == 1:
            body()
        else:
            # timing-only variant: hardware loop repeating identical work
            with tc.For_i(0, reps, 1):
                body()

    nc.compile()
    return nc


def _get_nc(reps=1):
    key = f"nc{reps}"
    if key not in _CACHE:
        _CACHE[key] = _build(reps)
    return _CACHE[key]


def _prep_inputs(t, z,
                 W_centres, W_log_sigmas, W_lin_w, W_lin_b,
                 U_centres, U_log_sigmas, U_lin_w, U_lin_b,
                 B_centres, B_log_sigmas, B_lin_w, B_lin_b):
    t = np.asarray(t, np.float32)
    Wt = _hypernet(t, np.asarray(W_centres, np.float32),
                   np.asarray(W_log_sigmas, np.float32),
                   np.asarray(W_lin_w, np.float32),
                   np.asarray(W_lin_b, np.float32)).reshape(WIDTH, D)
    Ut = _hypernet(t, np.asarray(U_centres, np.float32),
                   np.asarray(U_log_sigmas, np.float32),
                   np.asarray(U_lin_w, np.float32),
                   np.asarray(U_lin_b, np.float32)).reshape(WIDTH, D)
    Bt = _hypernet(t, np.asarray(B_centres, np.float32),
                   np.asarray(B_log_sigmas, np.float32),
                   np.asarray(B_lin_w, np.float32),
                   np.asarray(B_lin_b, np.float32))
    wu = np.sum(Wt * Ut, axis=1)

    U64 = (Ut / np.float32(WIDTH)).astype(np.float32)          # (64, 32)
    wu64 = (wu / np.float32(WIDTH)).astype(np.float32)         # (64,)

    import ml_dtypes
    bf = ml_dtypes.bfloat16

    WtT_hi, WtT_lo = _split_bf16(Wt.T.astype(np.float32))      # (32, 64) each
    # l1a (128, 128): pass1 = (z_hi + z_lo) @ W_hi for a quarter pair whose
    # rhs partitions are [q_hi(32), q'_hi(32), q_lo(32), q'_lo(32)]
    l1a = np.zeros((128, 128), np.float32)
    l1a[0:32, 0:64] = WtT_hi
    l1a[32:64, 64:128] = WtT_hi
    l1a[64:96, 0:64] = WtT_hi
    l1a[96:128, 64:128] = WtT_hi
    l1a = l1a.astype(bf)
    # l1b (64, 128): pass2 = z_hi @ W_lo
    l1b = np.zeros((64, 128), np.float32)
    l1b[0:32, 0:64] = WtT_lo
    l1b[32:64, 64:128] = WtT_lo
    l1b = l1b.astype(bf)

    blk = np.zeros((128, 64), np.float32)
    blk[0:64, 0:32] = U64
    blk[64:128, 32:64] = U64
    lhsT2 = np.concatenate([blk, blk], axis=1).astype(np.float16)  # (128, 128)
    lhsTg = np.zeros((128, 2), np.float32)
    lhsTg[0:64, 0] = wu64
    lhsTg[64:128, 1] = wu64
    lhsTg = lhsTg.astype(np.float16)
    bias = np.concatenate([Bt, Bt]).reshape(128, 1).astype(np.float32)

    z = np.ascontiguousarray(np.asarray(z, np.float32))
    # per-core packed transpose: (8, 4, 32768, 32) -> (8, 4, 32, 32768)
    zT = z.reshape(NCORES, 4, QUARTER, D).transpose(0, 1, 3, 2) \
          .reshape(NCORES, 128, QUARTER)
    z_hi, z_lo = _split_bf16(zT)
    # interleaved pair layouts: [q_hi, q'_hi, q_lo, q'_lo]
    z_il_a = np.concatenate(
        [z_hi[:, 0:64], z_lo[:, 0:64]], axis=1)                # (8, 128, Q)
    z_il_b = np.concatenate(
        [z_hi[:, 64:128], z_lo[:, 64:128]], axis=1)
    z_il_a = np.ascontiguousarray(z_il_a)
    z_il_b = np.ascontiguousarray(z_il_b)

    in_maps = [
        dict(z_il_a=z_il_a[c], z_il_b=z_il_b[c], l1a=l1a, l1b=l1b,
             lhsT2=lhsT2, lhsTg=lhsTg, bias=bias)
        for c in range(NCORES)
    ]
    return in_maps, wu


def _postprocess(results, wu):
    const = np.float32(np.sum(wu) / np.float32(WIDTH))
    dz_parts, dlp_parts = [], []
    for c in range(NCORES):
        dzT = results[c]["dzT"]                       # (128, 32768)
        dz = dzT.reshape(4, D, QUARTER).transpose(0, 2, 1).reshape(PER_CORE, D)
        dz_parts.append(dz)
        dlp = results[c]["dlp"].reshape(PER_CORE) - const
        dlp_parts.append(dlp)
    dz_dt = np.ascontiguousarray(np.concatenate(dz_parts, axis=0), dtype=np.float32)
    dlogp = np.concatenate(dlp_parts, axis=0).reshape(BATCH, 1).astype(np.float32)
    return dz_dt, dlogp


def kernel(t, z, logp_z, **params):
    from concourse.bass_utils import run_bass_kernel_spmd

    nc = _get_nc()
    in_maps, wu = _prep_inputs(t, z, **params)
    res = run_bass_kernel_spmd(nc, in_maps, core_ids=list(range(NCORES)))
    return _postprocess(res.results, wu)


# revision 28
# speedup vs baseline: 9.6036x; 2.9653x over previous
"""nn_CNF Trainium2 Bass kernel — 8-core data-parallel.

Math (per batch row b of z (B, 32)):
    h      = tanh(z @ Wt.T + Bt)            (B, 64)
    dz_dt  = (h @ Ut) / 64                  (B, 32)
    dlogp  = (h^2 @ wu - sum(wu)) / 64      (B, 1)   [= -(1-h^2)@wu/64]
Wt/Ut/Bt come from a tiny RBF hypernetwork of t (computed on host),
wu = sum(Wt*Ut, axis=1).

Device pipeline (per core, 131072 rows = 4 batch quarters x 64 superblocks
of 512 columns; software-pipelined so the in-order PE never waits):
  mm1 (bf16, exact hi/lo split of z and Wt done on host):
      z_il_[ab] (128, Q) = [q_hi, q'_hi, q_lo, q'_lo] partition blocks.
      pass1 = z @ W_hi (K=128), pass2 = z_hi @ W_lo (K=64) -> PSUM f32.
  ACT tanh+bias -> S_h (128, 1024) fp16;  DVE square -> S_g fp16.
  mm2h (fp16): one matmul/sb, N=1024, out (64,1024) fp16 at partition base
      64*(sb%2) of a shared 2-sb PSUM bank; DVE evacuates (128,1024)/2sb.
  mm2g (fp16): one matmul/sb, strips (2,1024) fp16 at base 64*(sb%2);
      DVE evacuates (66,1024)/2sb into an accumulator, 4 DMAs per 16 sb.
Outputs dzT/dlp are fp16 in a packed layout; host unpacks to f32.
"""
import numpy as np

NCORES = 8
BATCH = 1048576
D = 32
WIDTH = 64
PER_CORE = BATCH // NCORES      # 131072
QUARTER = PER_CORE // 4         # 32768
SB = 512                        # superblock columns
NSB = QUARTER // SB             # 64
ZCHUNK = 4096                   # z-in DMA columns (8 sbs, 1MB per tensor)
DZCHUNK = 8192                  # dz staging columns (8 sbs, 2MB fp16)
DLPGRP = 16                     # superblocks per dlogp flush

_CACHE = {}


def _hypernet(t, centres, log_sigmas, lin_w, lin_b):
    # matches reference._rbf_linear in float32
    d = np.abs(t - centres[:, 0]) / np.exp(log_sigmas)
    phi = np.exp(-(d * d)).astype(np.float32)
    return phi @ lin_w.T + lin_b


def _split_bf16(x):
    """x (f32) -> (hi, lo) bf16 with hi + lo ~= x (error ~2^-17 rel)."""
    import ml_dtypes
    hi = x.astype(ml_dtypes.bfloat16)
    lo = (x - hi.astype(np.float32)).astype(ml_dtypes.bfloat16)
    return hi, lo


def _build(reps=1, mm1_passes=2, with_mm2g=True):
    import concourse.bass as bass
    import concourse.tile as tile
    from concourse import bacc, mybir
    from contextlib import ExitStack

    f32 = mybir.dt.float32
    bf16 = mybir.dt.bfloat16
    fp16 = mybir.dt.float16
    AF = mybir.ActivationFunctionType

    nc = bacc.Bacc("TRN2", target_bir_lowering=False, debug=False,
                   enable_asserts=True, num_devices=NCORES)

    za_d = nc.dram_tensor("z_il_a", [128, QUARTER], bf16, kind="ExternalInput").ap()
    zb_d = nc.dram_tensor("z_il_b", [128, QUARTER], bf16, kind="ExternalInput").ap()
    l1a_d = nc.dram_tensor("l1a", [128, 128], bf16, kind="ExternalInput").ap()
    l1b_d = nc.dram_tensor("l1b", [64, 128], bf16, kind="ExternalInput").ap()
    l2_d = nc.dram_tensor("lhsT2", [128, 64], fp16, kind="ExternalInput").ap()
    lg_d = nc.dram_tensor("lhsTg", [128, 2], fp16, kind="ExternalInput").ap()
    b_d = nc.dram_tensor("bias", [128, 1], f32, kind="ExternalInput").ap()

    dzT_d = nc.dram_tensor("dzT", [128, QUARTER], fp16, kind="ExternalOutput").ap()
    dlp_d = nc.dram_tensor("dlp", [4, QUARTER], fp16, kind="ExternalOutput").ap()

    with tile.TileContext(nc) as tc, ExitStack() as ctx:
        const = ctx.enter_context(tc.tile_pool(name="const", bufs=1))
        zin = ctx.enter_context(tc.tile_pool(name="zin", bufs=2))
        shp = ctx.enter_context(tc.tile_pool(name="sh", bufs=2))
        sgp = ctx.enter_context(tc.tile_pool(name="sg", bufs=2))
        dzp = ctx.enter_context(tc.tile_pool(name="dz", bufs=2))
        dlpp = ctx.enter_context(tc.tile_pool(name="dlp", bufs=2))
        # PSUM: p1 2 banks x2 + p2 1 bank x2 + pg 1 bank x2 = 8 banks
        p1p = ctx.enter_context(tc.tile_pool(name="p1", bufs=2, space="PSUM"))
        p2p = ctx.enter_context(tc.tile_pool(name="p2", bufs=2, space="PSUM"))
        pgp = ctx.enter_context(tc.tile_pool(name="pg", bufs=2, space="PSUM"))

        l1a_t = const.tile([128, 128], bf16)
        nc.sync.dma_start(l1a_t[:], l1a_d[:])
        l1b_t = const.tile([64, 128], bf16)
        nc.sync.dma_start(l1b_t[:], l1b_d[:])
        l2_t = const.tile([128, 64], fp16)
        nc.sync.dma_start(l2_t[:], l2_d[:])
        lg_t = const.tile([128, 2], fp16)
        nc.sync.dma_start(lg_t[:], lg_d[:])
        b_t = const.tile([128, 1], f32)
        nc.sync.dma_start(b_t[:], b_d[:])

        state = {}
        p1s = {}
        shs = {}

        def body():
            # skew-1 software pipeline: PE work of sb m+1 (mm1) is emitted
            # before the PE work of sb m's tail (mm2h/mm2g).
            emit_mm1(0)
            for m in range(NSB):
                if m + 1 < NSB:
                    emit_mm1(m + 1)
                emit_tail(m)

        def emit_mm1(m):
            zn = ZCHUNK // SB
            if m % zn == 0:
                state["za"] = zin.tile([128, ZCHUNK], bf16, tag="za", name="za_t")
                nc.scalar.dma_start(state["za"][:],
                                    za_d[:, m * SB:m * SB + ZCHUNK])
                state["zb"] = zin.tile([128, ZCHUNK], bf16, tag="zb", name="zb_t")
                nc.scalar.dma_start(state["zb"][:],
                                    zb_d[:, m * SB:m * SB + ZCHUNK])
            c4 = (m % zn) * SB
            za = state["za"][:, c4:c4 + SB]
            zb = state["zb"][:, c4:c4 + SB]

            # mm1 per quarter-pair: z@W_hi (K=128, exact z) + z_hi@W_lo (K=64)
            p1 = p1p.tile([128, 2 * SB], f32)
            p1s[m] = p1
            for bank, zt in ((0, za), (1, zb)):
                sl = p1[:, bank * SB:(bank + 1) * SB]
                nc.tensor.matmul(sl, l1a_t[:], zt,
                                 start=True, stop=(mm1_passes == 1))
                if mm1_passes > 1:
                    nc.tensor.matmul(sl, l1b_t[:], zt[0:64, :],
                                     start=False, stop=True)

        def emit_tail(m):
            p1 = p1s.pop(m)
            sh = shp.tile([128, 2 * SB], fp16)
            nc.scalar.activation(sh[:], p1[:], AF.Tanh, bias=b_t[:], scale=1.0)
            sg = sgp.tile([128, 2 * SB], fp16)
            nc.vector.tensor_mul(sg[:], sh[:], sh[:])

            # mm2h: two matmuls -> dz.T packed (128, 512) f32
            p2 = p2p.tile([128, SB], f32, name="p2_t")
            nc.tensor.matmul(p2[0:64, :], l2_t[:], sh[:, 0:SB],
                             start=True, stop=True, tile_position=(0, 0))
            nc.tensor.matmul(p2[64:128, :], l2_t[:], sh[:, SB:2 * SB],
                             start=True, stop=True, tile_position=(0, 64))

            dn = DZCHUNK // SB
            if m % dn == 0:
                state["dzbig"] = dzp.tile([128, DZCHUNK], fp16, tag="dzbig",
                                          name="dzbig_t")
            dzbig = state["dzbig"]
            cd = (m % dn) * SB
            nc.vector.tensor_copy(dzbig[:, cd:cd + SB], p2[:])
            if m % dn == dn - 1:
                nc.sync.dma_start(dzT_d[:, (m - dn + 1) * SB:(m + 1) * SB],
                                  dzbig[:])

            # mm2g: strips (2, 512) f32; partition rotation over sb pairs
            if with_mm2g:
                off = 64 * (m % 2)
                if m % 2 == 0:
                    state["pg"] = pgp.tile([128, SB], f32, tag="pg",
                                           name="pg_t")
                pg = state["pg"]
                nc.tensor.matmul(pg[off + 0:off + 2, :], lg_t[:], sg[:, 0:SB],
                                 start=True, stop=True,
                                 tile_position=(0, off))
                nc.tensor.matmul(pg[off + 32:off + 34, :], lg_t[:],
                                 sg[:, SB:2 * SB], start=True, stop=True,
                                 tile_position=(0, off + 32))

            if m % 2 == 1 and with_mm2g:
                # strips layout in pg: {0,1}=even(q0,q1), {32,33}=even(q2,q3),
                # {64,65}=odd(q0,q1), {96,97}=odd(q2,q3)
                kg = (m % DLPGRP) // 2
                if kg == 0:
                    state["dlpacc"] = dlpp.tile(
                        [128, DLPGRP * SB // 2], fp16, tag="dlpacc",
                        name="dlpacc_t")
                dlpacc = state["dlpacc"]
                pg = state["pg"]
                nc.vector.tensor_copy(
                    dlpacc[0:98, kg * SB:(kg + 1) * SB], pg[0:98, :])
                if m % DLPGRP == DLPGRP - 1:
                    base = m - DLPGRP + 1
                    for (p0, r0, a) in ((0, 0, 0), (32, 2, 0),
                                        (64, 0, 1), (96, 2, 1)):
                        src = dlpacc[p0:p0 + 2, :] \
                            .rearrange("p (k n) -> p k n", n=SB)
                        dst = dlp_d[r0:r0 + 2,
                                    base * SB:(base + DLPGRP) * SB] \
                            .rearrange("p (k two n) -> p two k n",
                                       two=2, n=SB)[:, a]
                        nc.sync.dma_start(dst, src)

        if reps == 1:
            body()
        else:
            # timing-only variant: hardware loop repeating identical work
            with tc.For_i(0, reps, 1):
                body()

    nc.compile()
    return nc


def _get_nc(reps=1):
    key = f"nc{reps}"
    if key not in _CACHE:
        _CACHE[key] = _build(reps)
    return _CACHE[key]


def _prep_inputs(t, z,
                 W_centres, W_log_sigmas, W_lin_w, W_lin_b,
                 U_centres, U_log_sigmas, U_lin_w, U_lin_b,
                 B_centres, B_log_sigmas, B_lin_w, B_lin_b):
    t = np.asarray(t, np.float32)
    Wt = _hypernet(t, np.asarray(W_centres, np.float32),
                   np.asarray(W_log_sigmas, np.float32),
                   np.asarray(W_lin_w, np.float32),
                   np.asarray(W_lin_b, np.float32)).reshape(WIDTH, D)
    Ut = _hypernet(t, np.asarray(U_centres, np.float32),
                   np.asarray(U_log_sigmas, np.float32),
                   np.asarray(U_lin_w, np.float32),
                   np.asarray(U_lin_b, np.float32)).reshape(WIDTH, D)
    Bt = _hypernet(t, np.asarray(B_centres, np.float32),
                   np.asarray(B_log_sigmas, np.float32),
                   np.asarray(B_lin_w, np.float32),
                   np.asarray(B_lin_b, np.float32))
    wu = np.sum(Wt * Ut, axis=1)

    U64 = (Ut / np.float32(WIDTH)).astype(np.float32)          # (64, 32)
    wu64 = (wu / np.float32(WIDTH)).astype(np.float32)         # (64,)

    import ml_dtypes
    bf = ml_dtypes.bfloat16

    WtT_hi, WtT_lo = _split_bf16(Wt.T.astype(np.float32))      # (32, 64)
    l1a = np.zeros((128, 128), np.float32)
    l1a[0:32, 0:64] = WtT_hi
    l1a[32:64, 64:128] = WtT_hi
    l1a[64:96, 0:64] = WtT_hi
    l1a[96:128, 64:128] = WtT_hi
    l1a = l1a.astype(bf)
    l1b = np.zeros((64, 128), np.float32)
    l1b[0:32, 0:64] = WtT_lo
    l1b[32:64, 64:128] = WtT_lo
    l1b = l1b.astype(bf)

    lhsT2 = np.zeros((128, 64), np.float32)
    lhsT2[0:64, 0:32] = U64
    lhsT2[64:128, 32:64] = U64
    lhsT2 = lhsT2.astype(np.float16)
    lhsTg = np.zeros((128, 2), np.float32)
    lhsTg[0:64, 0] = wu64
    lhsTg[64:128, 1] = wu64
    lhsTg = lhsTg.astype(np.float16)
    bias = np.concatenate([Bt, Bt]).reshape(128, 1).astype(np.float32)

    z = np.ascontiguousarray(np.asarray(z, np.float32))
    # per-core packed transpose: (8, 4, 32768, 32) -> (8, 4, 32, 32768)
    zT = z.reshape(NCORES, 4, QUARTER, D).transpose(0, 1, 3, 2) \
          .reshape(NCORES, 128, QUARTER)
    z_hi, z_lo = _split_bf16(zT)
    z_il_a = np.ascontiguousarray(
        np.concatenate([z_hi[:, 0:64], z_lo[:, 0:64]], axis=1))
    z_il_b = np.ascontiguousarray(
        np.concatenate([z_hi[:, 64:128], z_lo[:, 64:128]], axis=1))

    in_maps = [
        dict(z_il_a=z_il_a[c], z_il_b=z_il_b[c], l1a=l1a, l1b=l1b,
             lhsT2=lhsT2, lhsTg=lhsTg, bias=bias)
        for c in range(NCORES)
    ]
    return in_maps, wu


def _postprocess(results, wu):
    const = np.float32(np.sum(wu) / np.float32(WIDTH))
    dz_parts, dlp_parts = [], []
    for c in range(NCORES):
        dzT = np.asarray(results[c]["dzT"]).astype(np.float32)  # (128, 32768)
        # partition 32q + d, col m*SB + n -> batch q*QUARTER + m*SB + n
        dz = dzT.reshape(4, D, QUARTER).transpose(0, 2, 1).reshape(PER_CORE, D)
        dz_parts.append(dz)
        dlp = np.asarray(results[c]["dlp"]).astype(np.float32) \
            .reshape(PER_CORE) - const
        dlp_parts.append(dlp)
    dz_dt = np.ascontiguousarray(np.concatenate(dz_parts, axis=0),
                                 dtype=np.float32)
    dlogp = np.concatenate(dlp_parts, axis=0).reshape(BATCH, 1) \
        .astype(np.float32)
    return dz_dt, dlogp


def kernel(t, z, logp_z, **params):
    from concourse.bass_utils import run_bass_kernel_spmd

    nc = _get_nc()
    in_maps, wu = _prep_inputs(t, z, **params)
    res = run_bass_kernel_spmd(nc, in_maps, core_ids=list(range(NCORES)))
    return _postprocess(res.results, wu)
